# revision 4
# baseline (speedup 1.0000x reference)
"""Trainium2 Bass kernel for nn_EnokeeEncoder (ragged mention pooling +
4-layer transformer + 50k-entity classifier), data-parallel over batch
across 8 NeuronCores.

Layout strategy per core (2 batches, 256 mention-tokens):
  - residual stream x: token-major [128 tokens/p, 768] f32 (LN/softmax easy)
  - matmul chains run feature-major via PE transposes of x
  - all big matmuls in bf16 (weights pre-cast+pre-transposed on host),
    f32 accumulation in PSUM; LN / softmax / residual in f32.
  - classifier bias folded into an augmented K=101 contraction row.
"""

import sys

for _p in ("/opt/trn_rl_repo",):
    if _p not in sys.path:
        sys.path.insert(0, _p)

import numpy as np
import ml_dtypes

BF16 = ml_dtypes.bfloat16

B, M, L, S = 16, 128, 32, 512
D, H, DFF, NL = 768, 12, 3072, 4
NE = 50000
HD = D // H
EPS = 1e-5
N_CORES = 8
BL = B // N_CORES          # batches per core
P = 128
KD = D // P                # 6 k-tiles over D
KF = DFF // P              # 24 k-tiles over DFF
MT = BL                    # token m-tiles per core (M == P)
NQK = 2 * D // P           # 12 m-tiles over q,k features

KERNEL_DEBUG = False
_CACHE = {}


def _build(cfg):
    (attn_b_val, vb_nz, outb_nz, ff2b_nz, ln1_nt, ln2_nt, debug) = cfg
    from contextlib import ExitStack

    import concourse.bass as bass
    import concourse.bacc as bacc
    import concourse.mybir as mybir
    import concourse.tile as tile
    from concourse.masks import make_identity

    dt = mybir.dt
    AF = mybir.ActivationFunctionType
    OP = mybir.AluOpType
    AX = mybir.AxisListType
    f32 = dt.float32
    bf16 = dt.bfloat16

    nc = bacc.Bacc("TRN2", target_bir_lowering=False, debug=False,
                   enable_asserts=False, num_devices=N_CORES)

    # ---- DRAM I/O ----
    lhs32_d = nc.dram_tensor("lhs32", [BL, L, D], f32, kind="ExternalInput").ap()
    vmT_d = nc.dram_tensor("vmT", [BL, L, M], f32, kind="ExternalInput").ap()
    attnw_d = nc.dram_tensor("attnw", [D], f32, kind="ExternalInput").ap()
    qkvw_d = nc.dram_tensor("qkvw", [NL, KD, P, 3 * D], bf16, kind="ExternalInput").ap()
    qkvb_d = nc.dram_tensor("qkvb", [NL, 3 * D], f32, kind="ExternalInput").ap()
    outw_d = nc.dram_tensor("outw", [NL, KD, P, D], bf16, kind="ExternalInput").ap()
    ff1w_d = nc.dram_tensor("ff1w", [NL, KD, P, DFF], bf16, kind="ExternalInput").ap()
    ff1b_d = nc.dram_tensor("ff1b", [NL, DFF], f32, kind="ExternalInput").ap()
    ff2w_d = nc.dram_tensor("ff2w", [NL, KF, P, D], bf16, kind="ExternalInput").ap()
    w1T_d = nc.dram_tensor("w1T", [KD, P, 100], bf16, kind="ExternalInput").ap()
    w2a_d = nc.dram_tensor("w2a", [101, NE], bf16, kind="ExternalInput").ap()
    outb_d = ff2b_d = None
    ln1w_d = ln1b_d = ln2w_d = ln2b_d = None
    if outb_nz:
        outb_d = nc.dram_tensor("outb", [NL, D], f32, kind="ExternalInput").ap()
    if ff2b_nz:
        ff2b_d = nc.dram_tensor("ff2b", [NL, D], f32, kind="ExternalInput").ap()
    if ln1_nt:
        ln1w_d = nc.dram_tensor("ln1w", [NL, D], f32, kind="ExternalInput").ap()
        ln1b_d = nc.dram_tensor("ln1b", [NL, D], f32, kind="ExternalInput").ap()
    if ln2_nt:
        ln2w_d = nc.dram_tensor("ln2w", [NL, D], f32, kind="ExternalInput").ap()
        ln2b_d = nc.dram_tensor("ln2b", [NL, D], f32, kind="ExternalInput").ap()
    out_d = nc.dram_tensor("out", [BL, M, NE], f32, kind="ExternalOutput").ap()
    xdbg_d = None
    if debug:
        xdbg_d = nc.dram_tensor("xdbg", [NL + 1, BL, M, D], f32,
                                kind="ExternalOutput").ap()

    def bcast_ap(ap, parts):
        return bass.AP(tensor=ap.tensor, offset=ap.offset,
                       ap=[[0, parts]] + [list(x) for x in ap.ap])

    with tile.TileContext(nc) as tc, ExitStack() as ctx:
        const = ctx.enter_context(tc.tile_pool(name="const", bufs=1))
        pools = ctx.enter_context(tc.tile_pool(name="pools", bufs=2))
        xpool = ctx.enter_context(tc.tile_pool(name="xpool", bufs=7))
        xTp = ctx.enter_context(tc.tile_pool(name="xTp", bufs=8))
        qkTp = ctx.enter_context(tc.tile_pool(name="qkTp", bufs=13))
        aoTp = ctx.enter_context(tc.tile_pool(name="aoTp", bufs=7))
        hTp = ctx.enter_context(tc.tile_pool(name="hTp", bufs=25))
        vp = ctx.enter_context(tc.tile_pool(name="vp", bufs=3))
        ap4 = ctx.enter_context(tc.tile_pool(name="ap4", bufs=4))
        stat = ctx.enter_context(tc.tile_pool(name="stat", bufs=8))
        wq = ctx.enter_context(tc.tile_pool(name="wq", bufs=6))
        wo = ctx.enter_context(tc.tile_pool(name="wo", bufs=7))
        wf1 = ctx.enter_context(tc.tile_pool(name="wf1", bufs=6))
        wf2 = ctx.enter_context(tc.tile_pool(name="wf2", bufs=5))
        w2p = ctx.enter_context(tc.tile_pool(name="w2p", bufs=6))
        ostp = ctx.enter_context(tc.tile_pool(name="ostp", bufs=3))
        psS = ctx.enter_context(tc.tile_pool(name="psS", bufs=2, space="PSUM"))
        psT = ctx.enter_context(tc.tile_pool(name="psT", bufs=2, space="PSUM"))
        psW = ctx.enter_context(tc.tile_pool(name="psW", bufs=2, space="PSUM"))

        # ---- constants ----
        idf = const.tile([P, P], f32, tag="idf", name="idf")
        make_identity(nc, idf[:])
        idb = const.tile([P, P], bf16, tag="idb", name="idb")
        make_identity(nc, idb[:])
        ones32 = const.tile([L, 1], f32, tag="ones32", name="ones32")
        nc.vector.memset(ones32[:], 1.0)
        epst = const.tile([P, 1], f32, tag="epst", name="epst")
        nc.vector.memset(epst[:], EPS)
        qkvb_sb = const.tile([P, NL, 2 * KD], f32, tag="qkvb", name="qkvb")
        ff1b_sb = const.tile([P, NL, KF], f32, tag="ff1b", name="ff1b")
        for i in range(NL):
            nc.gpsimd.dma_start(
                out=qkvb_sb[:, i, :],
                in_=qkvb_d[i, 0:2 * D].rearrange("(t p) -> p t", p=P))
            nc.gpsimd.dma_start(
                out=ff1b_sb[:, i, :],
                in_=ff1b_d[i].rearrange("(t p) -> p t", p=P))
        w1T_sb = const.tile([P, KD, 100], bf16, tag="w1T", name="w1T")
        for ko in range(KD):
            nc.sync.dma_start(out=w1T_sb[:, ko, :], in_=w1T_d[ko])

        # ---- mention pooling ----
        lhs32_sb = const.tile([L, BL, D], f32, tag="lhs32", name="lhs32")
        vmT_sb = const.tile([L, BL, M], f32, tag="vmT", name="vmT")
        for b in range(BL):
            nc.gpsimd.dma_start(out=lhs32_sb[:, b, :], in_=lhs32_d[b])
            nc.gpsimd.dma_start(out=vmT_sb[:, b, :], in_=vmT_d[b])
        attnw_sb = const.tile([L, D], f32, tag="attnw", name="attnw")
        nc.gpsimd.dma_start(out=attnw_sb[:], in_=bcast_ap(attnw_d, L))

        x_t = [xpool.tile([P, D], f32, tag="x", name="x") for _ in range(MT)]
        for b in range(BL):
            tmp = pools.tile([L, D], f32, tag="ptmp", name="ptmp")
            nc.vector.tensor_mul(tmp[:], lhs32_sb[:, b, :], attnw_sb[:])
            u = stat.tile([L, 1], f32, tag="u", name="u")
            nc.vector.tensor_reduce(u[:], tmp[:], axis=AX.X, op=OP.add)
            expT = pools.tile([L, M], f32, tag="pexp", name="pexp")
            nc.scalar.activation(expT[:], vmT_sb[:, b, :], AF.Exp,
                                 bias=float(attn_b_val), scale=u[:])
            wun = pools.tile([L, M], f32, tag="pwun", name="pwun")
            nc.vector.tensor_mul(wun[:], expT[:], vmT_sb[:, b, :])
            ps_d = psS.tile([P, 1], f32, tag="s", name="s")
            nc.tensor.matmul(ps_d[:], expT[:], ones32[:], start=True, stop=True)
            r = stat.tile([P, 1], f32, tag="r", name="r")
            nc.vector.reciprocal(r[:], ps_d[:])
            ps_x = psW.tile([P, D], f32, tag="w", name="w")
            for n0, n1 in ((0, 512), (512, D)):
                nc.tensor.matmul(ps_x[:, n0:n1], wun[:], lhs32_sb[:, b, n0:n1],
                                 start=True, stop=True)
            nc.scalar.activation(x_t[b][:], ps_x[:], AF.Copy, bias=0.0, scale=r[:])

        if debug:
            for b in range(BL):
                nc.sync.dma_start(out=xdbg_d[0, b], in_=x_t[b][:])

        # ---- helpers ----
        def transpose_cast(xt):
            """token-major f32 [128,768] x MT -> feature-major bf16 6x[128, 256]"""
            outs = [xTp.tile([P, P * MT], bf16, tag="xT", name="xT") for _ in range(KD)]
            for mo in range(MT):
                for ko in range(KD):
                    ps = psT.tile([P, P], f32, tag="t", name="t")
                    nc.tensor.transpose(ps[:], xt[mo][:, ko * P:(ko + 1) * P], idf[:])
                    nc.scalar.copy(outs[ko][:, mo * P:(mo + 1) * P], ps[:])
            return outs

        def layernorm(xin, w_bc, b_bc):
            xout = []
            for mo in range(MT):
                st = stat.tile([P, 3, 6], f32, tag="bns", name="bns")
                for s in range(3):
                    nc.vector.bn_stats(st[:, s, :], xin[mo][:, s * 256:(s + 1) * 256])
                mv = stat.tile([P, 2], f32, tag="mv", name="mv")
                nc.vector.bn_aggr(mv[:], st[:])
                std = stat.tile([P, 1], f32, tag="sd", name="sd")
                nc.scalar.activation(std[:], mv[:, 1:2], AF.Sqrt,
                                     bias=epst[:], scale=1.0)
                rstd = stat.tile([P, 1], f32, tag="rs", name="rs")
                nc.vector.reciprocal(rstd[:], std[:])
                nms = stat.tile([P, 1], f32, tag="ns", name="ns")
                nc.vector.tensor_scalar_mul(nms[:], mv[:, 0:1], -1.0)
                nc.vector.tensor_mul(nms[:], nms[:], rstd[:])
                xo = xpool.tile([P, D], f32, tag="x", name="x")
                nc.scalar.activation(xo[:], xin[mo][:], AF.Identity,
                                     bias=nms[:], scale=rstd[:])
                if w_bc is not None:
                    nc.vector.tensor_mul(xo[:], xo[:], w_bc[:])
                if b_bc is not None:
                    nc.vector.tensor_add(xo[:], xo[:], b_bc[:])
                xout.append(xo)
            return xout

        # ---- transformer layers ----
        for i in range(NL):
            qkvw_t = [wq.tile([P, 3 * D], bf16, tag="qkvw", name="qkvw") for _ in range(KD)]
            for ko in range(KD):
                nc.sync.dma_start(out=qkvw_t[ko][:], in_=qkvw_d[i, ko])
            outw_t = [wo.tile([P, D], bf16, tag="outw", name="outw") for _ in range(KD)]
            for ko in range(KD):
                nc.sync.dma_start(out=outw_t[ko][:], in_=outw_d[i, ko])
            ff1w_t = [wf1.tile([P, DFF], bf16, tag="ff1w", name="ff1w") for _ in range(KD)]
            for ko in range(KD):
                nc.sync.dma_start(out=ff1w_t[ko][:], in_=ff1w_d[i, ko])
            ff2w_t = [wf2.tile([P, D], bf16, tag="ff2w", name="ff2w") for _ in range(KF)]
            for ko in range(KF):
                nc.sync.dma_start(out=ff2w_t[ko][:], in_=ff2w_d[i, ko])
            vb_bc = None
            if vb_nz:
                vb_bc = pools.tile([P, D], f32, tag="vbb", name="vbb")
                nc.gpsimd.dma_start(out=vb_bc[:],
                                    in_=bcast_ap(qkvb_d[i, 2 * D:3 * D], P))
            outb_bc = None
            if outb_nz:
                outb_bc = pools.tile([P, D], f32, tag="obb", name="obb")
                nc.gpsimd.dma_start(out=outb_bc[:], in_=bcast_ap(outb_d[i], P))
            ff2b_bc = None
            if ff2b_nz:
                ff2b_bc = pools.tile([P, D], f32, tag="fbb", name="fbb")
                nc.gpsimd.dma_start(out=ff2b_bc[:], in_=bcast_ap(ff2b_d[i], P))
            ln1w_bc = ln1b_bc = ln2w_bc = ln2b_bc = None
            if ln1_nt:
                ln1w_bc = pools.tile([P, D], f32, tag="l1w", name="l1w")
                nc.gpsimd.dma_start(out=ln1w_bc[:], in_=bcast_ap(ln1w_d[i], P))
                ln1b_bc = pools.tile([P, D], f32, tag="l1b", name="l1b")
                nc.gpsimd.dma_start(out=ln1b_bc[:], in_=bcast_ap(ln1b_d[i], P))
            if ln2_nt:
                ln2w_bc = pools.tile([P, D], f32, tag="l2w", name="l2w")
                nc.gpsimd.dma_start(out=ln2w_bc[:], in_=bcast_ap(ln2w_d[i], P))
                ln2b_bc = pools.tile([P, D], f32, tag="l2b", name="l2b")
                nc.gpsimd.dma_start(out=ln2b_bc[:], in_=bcast_ap(ln2b_d[i], P))

            xT = transpose_cast(x_t)

            # q,k feature-major [1536, 256]
            qkT = []
            for mo12 in range(NQK):
                ps = psS.tile([P, P * MT], f32, tag="s", name="s")
                for ko in range(KD):
                    nc.tensor.matmul(ps[:], qkvw_t[ko][:, mo12 * P:(mo12 + 1) * P],
                                     xT[ko][:], start=(ko == 0), stop=(ko == KD - 1))
                t = qkTp.tile([P, P * MT], bf16, tag="qkT", name="qkT")
                nc.scalar.activation(t[:], ps[:], AF.Identity,
                                     bias=qkvb_sb[:, i, mo12:mo12 + 1], scale=1.0)
                qkT.append(t)

            # v token-major [256, 768]
            v_t = [vp.tile([P, D], bf16, tag="v", name="v") for _ in range(MT)]
            ps_v = [psW.tile([P, D], f32, tag="w", name="w") for _ in range(MT)]
            for ko in range(KD):
                for mo in range(MT):
                    for n0, n1 in ((0, 512), (512, D)):
                        nc.tensor.matmul(
                            ps_v[mo][:, n0:n1], xT[ko][:, mo * P:(mo + 1) * P],
                            qkvw_t[ko][:, 2 * D + n0:2 * D + n1],
                            start=(ko == 0), stop=(ko == KD - 1))
            for mo in range(MT):
                if vb_nz:
                    nc.vector.scalar_tensor_tensor(
                        v_t[mo][:], ps_v[mo][:], 1.0, vb_bc[:],
                        op0=OP.mult, op1=OP.add)
                else:
                    nc.scalar.copy(v_t[mo][:], ps_v[mo][:])

            # attention per (batch, head)
            aoT = [aoTp.tile([P, P * MT], bf16, tag="aoT", name="aoT") for _ in range(KD)]
            for b in range(MT):
                for h in range(H):
                    t_idx, row0 = h // 2, (h % 2) * HD
                    q_ap = qkT[t_idx][row0:row0 + HD, b * P:(b + 1) * P]
                    k_ap = qkT[KD + t_idx][row0:row0 + HD, b * P:(b + 1) * P]
                    ps_s = psS.tile([P, P], f32, tag="s", name="s")
                    nc.tensor.matmul(ps_s[:], q_ap, k_ap, start=True, stop=True)
                    ex = ap4.tile([P, P], f32, tag="exp", name="exp")
                    dsum = stat.tile([P, 1], f32, tag="ds", name="ds")
                    nc.scalar.activation(ex[:], ps_s[:], AF.Exp,
                                         scale=1.0 / np.sqrt(HD), accum_out=dsum[:])
                    rec = stat.tile([P, 1], f32, tag="rc", name="rc")
                    nc.vector.reciprocal(rec[:], dsum[:])
                    abf = ap4.tile([P, P], bf16, tag="abf", name="abf")
                    nc.vector.tensor_scalar_mul(abf[:], ex[:], rec[:])
                    ps_t = psT.tile([P, P], bf16, tag="t", name="t")
                    nc.tensor.transpose(ps_t[:], abf[:], idb[:])
                    aT = ap4.tile([P, P], bf16, tag="aT", name="aT")
                    nc.scalar.copy(aT[:], ps_t[:])
                    ps_ao = psS.tile([HD, P], f32, tag="s", name="s")
                    nc.tensor.matmul(ps_ao[:], v_t[b][:, h * HD:(h + 1) * HD],
                                     aT[:], start=True, stop=True)
                    nc.scalar.copy(aoT[t_idx][row0:row0 + HD, b * P:(b + 1) * P],
                                   ps_ao[:])

            # out-proj + residual
            ps_o = [psW.tile([P, D], f32, tag="w", name="w") for _ in range(MT)]
            for ko in range(KD):
                for mo in range(MT):
                    for n0, n1 in ((0, 512), (512, D)):
                        nc.tensor.matmul(
                            ps_o[mo][:, n0:n1], aoT[ko][:, mo * P:(mo + 1) * P],
                            outw_t[ko][:, n0:n1],
                            start=(ko == 0), stop=(ko == KD - 1))
            x1_t = [xpool.tile([P, D], f32, tag="x", name="x") for _ in range(MT)]
            for mo in range(MT):
                nc.vector.scalar_tensor_tensor(
                    x1_t[mo][:], ps_o[mo][:], 1.0, x_t[mo][:],
                    op0=OP.mult, op1=OP.add)
                if outb_nz:
                    nc.vector.tensor_add(x1_t[mo][:], x1_t[mo][:], outb_bc[:])

            x1n_t = layernorm(x1_t, ln1w_bc, ln1b_bc)
            x1nT = transpose_cast(x1n_t)

            # ff1 (relu) feature-major [3072, 256]
            hT = [hTp.tile([P, P * MT], bf16, tag="hT", name="hT") for _ in range(KF)]
            for mo24 in range(KF):
                ps = psS.tile([P, P * MT], f32, tag="s", name="s")
                for ko in range(KD):
                    nc.tensor.matmul(ps[:], ff1w_t[ko][:, mo24 * P:(mo24 + 1) * P],
                                     x1nT[ko][:], start=(ko == 0), stop=(ko == KD - 1))
                nc.scalar.activation(hT[mo24][:], ps[:], AF.Relu,
                                     bias=ff1b_sb[:, i, mo24:mo24 + 1], scale=1.0)

            # ff2 + residual
            ps_y = [psW.tile([P, D], f32, tag="w", name="w") for _ in range(MT)]
            for ko in range(KF):
                for mo in range(MT):
                    for n0, n1 in ((0, 512), (512, D)):
                        nc.tensor.matmul(
                            ps_y[mo][:, n0:n1], hT[ko][:, mo * P:(mo + 1) * P],
                            ff2w_t[ko][:, n0:n1],
                            start=(ko == 0), stop=(ko == KF - 1))
            x2_t = [xpool.tile([P, D], f32, tag="x", name="x") for _ in range(MT)]
            for mo in range(MT):
                nc.vector.scalar_tensor_tensor(
                    x2_t[mo][:], ps_y[mo][:], 1.0, x1n_t[mo][:],
                    op0=OP.mult, op1=OP.add)
                if ff2b_nz:
                    nc.vector.tensor_add(x2_t[mo][:], x2_t[mo][:], ff2b_bc[:])

            x_t = layernorm(x2_t, ln2w_bc, ln2b_bc)
            if debug:
                for b in range(BL):
                    nc.sync.dma_start(out=xdbg_d[i + 1, b], in_=x_t[b][:])

        # ---- classifier ----
        xT = transpose_cast(x_t)
        ps_h = psS.tile([P, P * MT], f32, tag="s", name="s")
        for ko in range(KD):
            nc.tensor.matmul(ps_h[0:100, :], w1T_sb[:, ko, :], xT[ko][:],
                             start=(ko == 0), stop=(ko == KD - 1))
        hTa = const.tile([P, P * MT], bf16, tag="hTa", name="hTa")
        nc.vector.memset(hTa[:, :], 1.0)
        nc.scalar.copy(hTa[0:100, :], ps_h[0:100, :])

        CH = 1024
        for c0 in range(0, NE, CH):
            cw = min(CH, NE - c0)
            w2t = w2p.tile([101, CH], bf16, tag="w2", name="w2")
            nc.sync.dma_start(out=w2t[:, 0:cw], in_=w2a_d[:, c0:c0 + cw])
            ost = [ostp.tile([P, CH], f32, tag="ost", name="ost") for _ in range(MT)]
            for s0 in range(0, cw, 512):
                sw = min(512, cw - s0)
                for mo in range(MT):
                    ps = psS.tile([P, 512], f32, tag="s", name="s")
                    nc.tensor.matmul(ps[:, 0:sw],
                                     hTa[0:101, mo * P:(mo + 1) * P],
                                     w2t[0:101, s0:s0 + sw], start=True, stop=True)
                    nc.scalar.copy(ost[mo][:, s0:s0 + sw], ps[:, 0:sw])
            for mo in range(MT):
                nc.sync.dma_start(out=out_d[mo, :, c0:c0 + cw], in_=ost[mo][:, 0:cw])

    nc.compile()
    return nc


def _prep(inputs):
    lhs = np.asarray(inputs["last_hidden_state"], dtype=np.float32)
    pos = np.asarray(inputs["entity_position_ids"])
    msk = np.asarray(inputs["entity_attention_mask"])
    qkv_w = np.asarray(inputs["qkv_w"], dtype=np.float32)
    qkv_b = np.asarray(inputs["qkv_b"], dtype=np.float32)
    out_w = np.asarray(inputs["out_w"], dtype=np.float32)
    out_b = np.asarray(inputs["out_b"], dtype=np.float32)
    ln1_w = np.asarray(inputs["ln1_w"], dtype=np.float32)
    ln1_b = np.asarray(inputs["ln1_b"], dtype=np.float32)
    ff1_w = np.asarray(inputs["ff1_w"], dtype=np.float32)
    ff1_b = np.asarray(inputs["ff1_b"], dtype=np.float32)
    ff2_w = np.asarray(inputs["ff2_w"], dtype=np.float32)
    ff2_b = np.asarray(inputs["ff2_b"], dtype=np.float32)
    ln2_w = np.asarray(inputs["ln2_w"], dtype=np.float32)
    ln2_b = np.asarray(inputs["ln2_b"], dtype=np.float32)
    cls_w1 = np.asarray(inputs["cls_w1"], dtype=np.float32)
    cls_w2 = np.asarray(inputs["cls_w2"], dtype=np.float32)
    cls_b2 = np.asarray(inputs["cls_b2"], dtype=np.float32)
    attn_w = np.asarray(inputs["attn_w"], dtype=np.float32)
    attn_b = float(np.asarray(inputs["attn_b"], dtype=np.float32))

    # ragged valid mask: 1 up to the first -1 (and only where attention mask set)
    nb = np.cumprod((pos != -1).astype(np.int32), axis=-1)
    valid = (msk != 0).astype(np.int32)[:, :, None] * nb       # [B, M, L]
    vmT = np.ascontiguousarray(valid.transpose(0, 2, 1)).astype(np.float32)

    cfg = (
        attn_b,
        bool(np.any(qkv_b[:, 2 * D:])),
        bool(np.any(out_b)),
        bool(np.any(ff2_b)),
        not (np.all(ln1_w == 1.0) and np.all(ln1_b == 0.0)),
        not (np.all(ln2_w == 1.0) and np.all(ln2_b == 0.0)),
        bool(KERNEL_DEBUG),
    )

    shared = {
        "attnw": attn_w,
        "qkvw": np.ascontiguousarray(qkv_w.transpose(0, 2, 1)).reshape(
            NL, KD, P, 3 * D).astype(BF16),
        "qkvb": qkv_b,
        "outw": np.ascontiguousarray(out_w.transpose(0, 2, 1)).reshape(
            NL, KD, P, D).astype(BF16),
        "ff1w": np.ascontiguousarray(ff1_w.transpose(0, 2, 1)).reshape(
            NL, KD, P, DFF).astype(BF16),
        "ff1b": ff1_b,
        "ff2w": np.ascontiguousarray(ff2_w.transpose(0, 2, 1)).reshape(
            NL, KF, P, D).astype(BF16),
        "w1T": np.ascontiguousarray(cls_w1.T).reshape(KD, P, 100).astype(BF16),
        "w2a": np.concatenate(
            [cls_w2.T, cls_b2[None, :]], axis=0).astype(BF16),
    }
    if cfg[2]:
        shared["outb"] = out_b
    if cfg[3]:
        shared["ff2b"] = ff2_b
    if cfg[4]:
        shared["ln1w"] = ln1_w
        shared["ln1b"] = ln1_b
    if cfg[5]:
        shared["ln2w"] = ln2_w
        shared["ln2b"] = ln2_b

    lhs32 = np.ascontiguousarray(lhs[:, :L, :])
    in_maps = []
    for c in range(N_CORES):
        m = dict(shared)
        m["lhs32"] = np.ascontiguousarray(lhs32[c * BL:(c + 1) * BL])
        m["vmT"] = np.ascontiguousarray(vmT[c * BL:(c + 1) * BL])
        in_maps.append(m)
    return cfg, in_maps


def kernel(**inputs):
    from concourse.bass_utils import run_bass_kernel_spmd

    cfg, in_maps = _prep(inputs)
    if cfg not in _CACHE:
        _CACHE[cfg] = _build(cfg)
    nc = _CACHE[cfg]
    res = run_bass_kernel_spmd(nc, in_maps, core_ids=list(range(N_CORES)))
    out = np.concatenate([res.results[c]["out"] for c in range(N_CORES)], axis=0)
    if KERNEL_DEBUG:
        kernel.last_debug = [res.results[c].get("xdbg") for c in range(N_CORES)]
    return out


# revision 10
# speedup vs baseline: 1.5507x; 1.5507x over previous
"""Trainium2 Bass kernel for nn_EnokeeEncoder (ragged mention pooling +
4-layer transformer + 50k-entity classifier), data-parallel over batch
across 8 NeuronCores.

Layout strategy per core (2 batches, 256 mention-tokens):
  - residual stream x: token-major [128 tokens/p, 768] f32 (LN/softmax easy)
  - matmul chains run feature-major via PE transposes of x
  - all big matmuls in bf16 (weights pre-cast+pre-transposed on host),
    f32 accumulation in PSUM; LN / softmax / residual in f32.
  - classifier bias folded into an augmented K=101 contraction row.
"""

import sys

for _p in ("/opt/trn_rl_repo",):
    if _p not in sys.path:
        sys.path.insert(0, _p)

import numpy as np
import ml_dtypes

BF16 = ml_dtypes.bfloat16

B, M, L, S = 16, 128, 32, 512
D, H, DFF, NL = 768, 12, 3072, 4
NE = 50000
HD = D // H
EPS = 1e-5
N_CORES = 8
BL = B // N_CORES          # batches per core
P = 128
KD = D // P                # 6 k-tiles over D
KF = DFF // P              # 24 k-tiles over DFF
MT = BL                    # token m-tiles per core (M == P)
NQK = 2 * D // P           # 12 m-tiles over q,k features

KERNEL_DEBUG = False
_CACHE = {}


def _build(cfg):
    (attn_b_val, vb_nz, outb_nz, ff2b_nz, ln1_nt, ln2_nt, debug) = cfg
    from contextlib import ExitStack

    import concourse.bass as bass
    import concourse.bacc as bacc
    import concourse.mybir as mybir
    import concourse.tile as tile
    from concourse.masks import make_identity

    dt = mybir.dt
    AF = mybir.ActivationFunctionType
    OP = mybir.AluOpType
    AX = mybir.AxisListType
    f32 = dt.float32
    bf16 = dt.bfloat16

    nc = bacc.Bacc("TRN2", target_bir_lowering=False, debug=False,
                   enable_asserts=False, num_devices=N_CORES)

    # ---- DRAM I/O ----
    lhs32_d = nc.dram_tensor("lhs32", [BL, L, D], f32, kind="ExternalInput").ap()
    vmT_d = nc.dram_tensor("vmT", [BL, L, M], f32, kind="ExternalInput").ap()
    attnw_d = nc.dram_tensor("attnw", [D], f32, kind="ExternalInput").ap()
    qkvw_d = nc.dram_tensor("qkvw", [NL, KD, P, 3 * D], bf16, kind="ExternalInput").ap()
    qkvb_d = nc.dram_tensor("qkvb", [NL, 3 * D], f32, kind="ExternalInput").ap()
    outw_d = nc.dram_tensor("outw", [NL, KD, P, D], bf16, kind="ExternalInput").ap()
    ff1w_d = nc.dram_tensor("ff1w", [NL, KD, P, DFF], bf16, kind="ExternalInput").ap()
    ff1b_d = nc.dram_tensor("ff1b", [NL, DFF], f32, kind="ExternalInput").ap()
    ff2w_d = nc.dram_tensor("ff2w", [NL, KF, P, D], bf16, kind="ExternalInput").ap()
    w1T_d = nc.dram_tensor("w1T", [KD, P, 100], bf16, kind="ExternalInput").ap()
    NCH = (NE + 767) // 768
    w2a_d = nc.dram_tensor("w2a", [NCH, P, 768], bf16, kind="ExternalInput").ap()
    outb_d = ff2b_d = None
    ln1w_d = ln1b_d = ln2w_d = ln2b_d = None
    if outb_nz:
        outb_d = nc.dram_tensor("outb", [NL, D], f32, kind="ExternalInput").ap()
    if ff2b_nz:
        ff2b_d = nc.dram_tensor("ff2b", [NL, D], f32, kind="ExternalInput").ap()
    if ln1_nt:
        ln1w_d = nc.dram_tensor("ln1w", [NL, D], f32, kind="ExternalInput").ap()
        ln1b_d = nc.dram_tensor("ln1b", [NL, D], f32, kind="ExternalInput").ap()
    if ln2_nt:
        ln2w_d = nc.dram_tensor("ln2w", [NL, D], f32, kind="ExternalInput").ap()
        ln2b_d = nc.dram_tensor("ln2b", [NL, D], f32, kind="ExternalInput").ap()
    out_d = nc.dram_tensor("out", [BL, M, NE], f32, kind="ExternalOutput").ap()
    xdbg_d = None
    if debug:
        xdbg_d = nc.dram_tensor("xdbg", [NL + 1, BL, M, D], f32,
                                kind="ExternalOutput").ap()

    def bcast_ap(ap, parts):
        return bass.AP(tensor=ap.tensor, offset=ap.offset,
                       ap=[[0, parts]] + [list(x) for x in ap.ap])

    def evict_copy(idx, out_ap, in_ap):
        if idx % 2 == 0:
            nc.scalar.copy(out_ap, in_ap)
        else:
            nc.vector.tensor_copy(out_ap, in_ap)

    with tile.TileContext(nc) as tc, ExitStack() as ctx:
        const = ctx.enter_context(tc.tile_pool(name="const", bufs=1))
        pools = ctx.enter_context(tc.tile_pool(name="pools", bufs=2))
        xpool = ctx.enter_context(tc.tile_pool(name="xpool", bufs=7))
        xTp = ctx.enter_context(tc.tile_pool(name="xTp", bufs=8))
        qkTp = ctx.enter_context(tc.tile_pool(name="qkTp", bufs=13))
        aoTp = ctx.enter_context(tc.tile_pool(name="aoTp", bufs=7))
        hTp = ctx.enter_context(tc.tile_pool(name="hTp", bufs=25))
        vp = ctx.enter_context(tc.tile_pool(name="vp", bufs=3))
        ap4 = ctx.enter_context(tc.tile_pool(name="ap4", bufs=4))
        stat = ctx.enter_context(tc.tile_pool(name="stat", bufs=8))
        wq = ctx.enter_context(tc.tile_pool(name="wq", bufs=6))
        wo = ctx.enter_context(tc.tile_pool(name="wo", bufs=7))
        wf1 = ctx.enter_context(tc.tile_pool(name="wf1", bufs=6))
        wf2 = ctx.enter_context(tc.tile_pool(name="wf2", bufs=5))
        w2p = ctx.enter_context(tc.tile_pool(name="w2p", bufs=6))
        ostp = ctx.enter_context(tc.tile_pool(name="ostp", bufs=6))
        psS = ctx.enter_context(tc.tile_pool(name="psS", bufs=2, space="PSUM"))
        psT = ctx.enter_context(tc.tile_pool(name="psT", bufs=2, space="PSUM"))
        psW = ctx.enter_context(tc.tile_pool(name="psW", bufs=2, space="PSUM"))

        # ---- constants ----
        idf = const.tile([P, P], f32, tag="idf", name="idf")
        make_identity(nc, idf[:])
        idb = const.tile([P, P], bf16, tag="idb", name="idb")
        make_identity(nc, idb[:])
        ones32 = const.tile([L, 1], f32, tag="ones32", name="ones32")
        nc.vector.memset(ones32[:], 1.0)
        epst = const.tile([P, 1], f32, tag="epst", name="epst")
        nc.vector.memset(epst[:], EPS)
        onesc = const.tile([P, 1], bf16, tag="onesc", name="onesc")
        nc.vector.memset(onesc[:], 1.0)
        qkvb_sb = const.tile([P, NL, 2 * KD], f32, tag="qkvb", name="qkvb")
        ff1b_sb = const.tile([P, NL, KF], f32, tag="ff1b", name="ff1b")
        for i in range(NL):
            nc.gpsimd.dma_start(
                out=qkvb_sb[:, i, :],
                in_=qkvb_d[i, 0:2 * D].rearrange("(t p) -> p t", p=P))
            nc.gpsimd.dma_start(
                out=ff1b_sb[:, i, :],
                in_=ff1b_d[i].rearrange("(t p) -> p t", p=P))
        w1T_sb = const.tile([P, KD, 100], bf16, tag="w1T", name="w1T")
        for ko in range(KD):
            nc.sync.dma_start(out=w1T_sb[:, ko, :], in_=w1T_d[ko])

        # ---- mention pooling ----
        lhs32_sb = const.tile([L, BL, D], f32, tag="lhs32", name="lhs32")
        vmT_sb = const.tile([L, BL, M], f32, tag="vmT", name="vmT")
        for b in range(BL):
            nc.gpsimd.dma_start(out=lhs32_sb[:, b, :], in_=lhs32_d[b])
            nc.gpsimd.dma_start(out=vmT_sb[:, b, :], in_=vmT_d[b])
        attnw_sb = const.tile([L, D], f32, tag="attnw", name="attnw")
        nc.gpsimd.dma_start(out=attnw_sb[:], in_=bcast_ap(attnw_d, L))

        x_t = [xpool.tile([P, D], f32, tag="x", name="x") for _ in range(MT)]
        for b in range(BL):
            tmp = pools.tile([L, D], f32, tag="ptmp", name="ptmp")
            nc.vector.tensor_mul(tmp[:], lhs32_sb[:, b, :], attnw_sb[:])
            u = stat.tile([L, 1], f32, tag="u", name="u")
            nc.vector.tensor_reduce(u[:], tmp[:], axis=AX.X, op=OP.add)
            expT = pools.tile([L, M], f32, tag="pexp", name="pexp")
            nc.scalar.activation(expT[:], vmT_sb[:, b, :], AF.Exp,
                                 bias=float(attn_b_val), scale=u[:])
            wun = pools.tile([L, M], f32, tag="pwun", name="pwun")
            nc.vector.tensor_mul(wun[:], expT[:], vmT_sb[:, b, :])
            ps_d = psS.tile([P, 1], f32, tag="s", name="s")
            nc.tensor.matmul(ps_d[:], expT[:], ones32[:], start=True, stop=True)
            r = stat.tile([P, 1], f32, tag="r", name="r")
            nc.vector.reciprocal(r[:], ps_d[:])
            ps_x = psW.tile([P, D], f32, tag="w", name="w")
            for n0, n1 in ((0, 512), (512, D)):
                nc.tensor.matmul(ps_x[:, n0:n1], wun[:], lhs32_sb[:, b, n0:n1],
                                 start=True, stop=True)
            nc.vector.tensor_scalar_mul(x_t[b][:], ps_x[:], r[:])

        if debug:
            for b in range(BL):
                nc.sync.dma_start(out=xdbg_d[0, b], in_=x_t[b][:])

        # ---- helpers ----
        def transpose_cast(xt):
            """token-major f32 [128,768] x MT -> feature-major bf16 6x[128, 256]"""
            outs = [xTp.tile([P, P * MT], bf16, tag="xT", name="xT") for _ in range(KD)]
            for mo in range(MT):
                for ko in range(KD):
                    ps = psT.tile([P, P], f32, tag="t", name="t")
                    nc.tensor.transpose(ps[:], xt[mo][:, ko * P:(ko + 1) * P], idf[:])
                    evict_copy(ko, outs[ko][:, mo * P:(mo + 1) * P], ps[:])
            return outs

        def layernorm(xin, w_bc, b_bc):
            xout = []
            for mo in range(MT):
                st = stat.tile([P, 3, 6], f32, tag="bns", name="bns")
                for s in range(3):
                    nc.vector.bn_stats(st[:, s, :], xin[mo][:, s * 256:(s + 1) * 256])
                mv = stat.tile([P, 2], f32, tag="mv", name="mv")
                nc.vector.bn_aggr(mv[:], st[:])
                std = stat.tile([P, 1], f32, tag="sd", name="sd")
                nc.scalar.activation(std[:], mv[:, 1:2], AF.Sqrt,
                                     bias=epst[:], scale=1.0)
                rstd = stat.tile([P, 1], f32, tag="rs", name="rs")
                nc.vector.reciprocal(rstd[:], std[:])
                nms = stat.tile([P, 1], f32, tag="ns", name="ns")
                nc.vector.tensor_scalar_mul(nms[:], mv[:, 0:1], -1.0)
                nc.vector.tensor_mul(nms[:], nms[:], rstd[:])
                xo = xpool.tile([P, D], f32, tag="x", name="x")
                nc.vector.tensor_scalar(xo[:], xin[mo][:], rstd[:], nms[:],
                                        op0=OP.mult, op1=OP.add)
                if w_bc is not None:
                    nc.vector.tensor_mul(xo[:], xo[:], w_bc[:])
                if b_bc is not None:
                    nc.vector.tensor_add(xo[:], xo[:], b_bc[:])
                xout.append(xo)
            return xout

        # ---- transformer layers ----
        for i in range(NL):
            qkvw_t = [wq.tile([P, 3 * D], bf16, tag="qkvw", name="qkvw") for _ in range(KD)]
            for ko in range(KD):
                nc.sync.dma_start(out=qkvw_t[ko][:], in_=qkvw_d[i, ko])
            outw_t = [wo.tile([P, D], bf16, tag="outw", name="outw") for _ in range(KD)]
            for ko in range(KD):
                nc.sync.dma_start(out=outw_t[ko][:], in_=outw_d[i, ko])
            ff1w_t = [wf1.tile([P, DFF], bf16, tag="ff1w", name="ff1w") for _ in range(KD)]
            for ko in range(KD):
                nc.sync.dma_start(out=ff1w_t[ko][:], in_=ff1w_d[i, ko])
            ff2w_t = [wf2.tile([P, D], bf16, tag="ff2w", name="ff2w") for _ in range(KF)]
            for ko in range(KF):
                nc.sync.dma_start(out=ff2w_t[ko][:], in_=ff2w_d[i, ko])
            vb_bc = None
            if vb_nz:
                vb_bc = pools.tile([P, D], f32, tag="vbb", name="vbb")
                nc.gpsimd.dma_start(out=vb_bc[:],
                                    in_=bcast_ap(qkvb_d[i, 2 * D:3 * D], P))
            outb_bc = None
            if outb_nz:
                outb_bc = pools.tile([P, D], f32, tag="obb", name="obb")
                nc.gpsimd.dma_start(out=outb_bc[:], in_=bcast_ap(outb_d[i], P))
            ff2b_bc = None
            if ff2b_nz:
                ff2b_bc = pools.tile([P, D], f32, tag="fbb", name="fbb")
                nc.gpsimd.dma_start(out=ff2b_bc[:], in_=bcast_ap(ff2b_d[i], P))
            ln1w_bc = ln1b_bc = ln2w_bc = ln2b_bc = None
            if ln1_nt:
                ln1w_bc = pools.tile([P, D], f32, tag="l1w", name="l1w")
                nc.gpsimd.dma_start(out=ln1w_bc[:], in_=bcast_ap(ln1w_d[i], P))
                ln1b_bc = pools.tile([P, D], f32, tag="l1b", name="l1b")
                nc.gpsimd.dma_start(out=ln1b_bc[:], in_=bcast_ap(ln1b_d[i], P))
            if ln2_nt:
                ln2w_bc = pools.tile([P, D], f32, tag="l2w", name="l2w")
                nc.gpsimd.dma_start(out=ln2w_bc[:], in_=bcast_ap(ln2w_d[i], P))
                ln2b_bc = pools.tile([P, D], f32, tag="l2b", name="l2b")
                nc.gpsimd.dma_start(out=ln2b_bc[:], in_=bcast_ap(ln2b_d[i], P))

            xT = transpose_cast(x_t)

            # q,k feature-major [1536, 256]
            qkT = []
            for mo12 in range(NQK):
                ps = psS.tile([P, P * MT], f32, tag="s", name="s")
                for ko in range(KD):
                    nc.tensor.matmul(ps[:], qkvw_t[ko][:, mo12 * P:(mo12 + 1) * P],
                                     xT[ko][:], start=(ko == 0), stop=(ko == KD - 1))
                t = qkTp.tile([P, P * MT], bf16, tag="qkT", name="qkT")
                if mo12 % 2 == 0:
                    nc.scalar.activation(t[:], ps[:], AF.Identity,
                                         bias=qkvb_sb[:, i, mo12:mo12 + 1], scale=1.0)
                else:
                    nc.vector.tensor_scalar_add(t[:], ps[:],
                                                qkvb_sb[:, i, mo12:mo12 + 1])
                qkT.append(t)

            # v token-major [256, 768]
            v_t = [vp.tile([P, D], bf16, tag="v", name="v") for _ in range(MT)]
            ps_v = [psW.tile([P, D], f32, tag="w", name="w") for _ in range(MT)]
            for ko in range(KD):
                for mo in range(MT):
                    for n0, n1 in ((0, 512), (512, D)):
                        nc.tensor.matmul(
                            ps_v[mo][:, n0:n1], xT[ko][:, mo * P:(mo + 1) * P],
                            qkvw_t[ko][:, 2 * D + n0:2 * D + n1],
                            start=(ko == 0), stop=(ko == KD - 1))
            for mo in range(MT):
                if vb_nz:
                    nc.vector.scalar_tensor_tensor(
                        v_t[mo][:], ps_v[mo][:], 1.0, vb_bc[:],
                        op0=OP.mult, op1=OP.add)
                else:
                    nc.vector.tensor_copy(v_t[mo][:], ps_v[mo][:])

            # attention per (batch, head): scores -> exp(bf16) -> transpose
            # -> AV (token-major) with an extra ones-column giving the softmax
            # denominator; normalization folded into the ao eviction.
            ao_bf = [vp.tile([P, D], bf16, tag="ao", name="ao") for _ in range(MT)]
            for b in range(MT):
                for h in range(H):
                    t_idx, row0 = h // 2, (h % 2) * HD
                    q_ap = qkT[t_idx][row0:row0 + HD, b * P:(b + 1) * P]
                    k_ap = qkT[KD + t_idx][row0:row0 + HD, b * P:(b + 1) * P]
                    ps_s = psS.tile([P, P], f32, tag="s", name="s")
                    nc.tensor.matmul(ps_s[:], q_ap, k_ap, start=True, stop=True)
                    ex = ap4.tile([P, P], bf16, tag="abf", name="abf")
                    nc.scalar.activation(ex[:], ps_s[:], AF.Exp,
                                         scale=1.0 / np.sqrt(HD))
                    ps_t = psT.tile([P, P], bf16, tag="t", name="t")
                    nc.tensor.transpose(ps_t[:], ex[:], idb[:])
                    aT = ap4.tile([P, P], bf16, tag="aT", name="aT")
                    evict_copy(h, aT[:], ps_t[:])
                    ps_ao = psS.tile([P, HD + 1], f32, tag="s", name="s")
                    nc.tensor.matmul(ps_ao[:, 0:HD], aT[:],
                                     v_t[b][:, h * HD:(h + 1) * HD],
                                     start=True, stop=True)
                    nc.tensor.matmul(ps_ao[:, HD:HD + 1], aT[:], onesc[:],
                                     start=True, stop=True)
                    rec = stat.tile([P, 1], f32, tag="rc", name="rc")
                    nc.vector.reciprocal(rec[:], ps_ao[:, HD:HD + 1])
                    nc.vector.tensor_scalar_mul(
                        ao_bf[b][:, h * HD:(h + 1) * HD], ps_ao[:, 0:HD], rec[:])

            # transpose ao to feature-major for the out-projection
            aoT = [aoTp.tile([P, P * MT], bf16, tag="aoT", name="aoT") for _ in range(KD)]
            for mo in range(MT):
                for ko in range(KD):
                    ps = psT.tile([P, P], bf16, tag="t", name="t")
                    nc.tensor.transpose(ps[:], ao_bf[mo][:, ko * P:(ko + 1) * P],
                                        idb[:])
                    evict_copy(ko + 1, aoT[ko][:, mo * P:(mo + 1) * P], ps[:])

            # out-proj + residual
            ps_o = [psW.tile([P, D], f32, tag="w", name="w") for _ in range(MT)]
            for ko in range(KD):
                for mo in range(MT):
                    for n0, n1 in ((0, 512), (512, D)):
                        nc.tensor.matmul(
                            ps_o[mo][:, n0:n1], aoT[ko][:, mo * P:(mo + 1) * P],
                            outw_t[ko][:, n0:n1],
                            start=(ko == 0), stop=(ko == KD - 1))
            x1_t = [xpool.tile([P, D], f32, tag="x", name="x") for _ in range(MT)]
            for mo in range(MT):
                nc.vector.scalar_tensor_tensor(
                    x1_t[mo][:], ps_o[mo][:], 1.0, x_t[mo][:],
                    op0=OP.mult, op1=OP.add)
                if outb_nz:
                    nc.vector.tensor_add(x1_t[mo][:], x1_t[mo][:], outb_bc[:])

            x1n_t = layernorm(x1_t, ln1w_bc, ln1b_bc)
            x1nT = transpose_cast(x1n_t)

            # ff1 (relu) feature-major [3072, 256]
            hT = [hTp.tile([P, P * MT], bf16, tag="hT", name="hT") for _ in range(KF)]
            for mo24 in range(KF):
                ps = psS.tile([P, P * MT], f32, tag="s", name="s")
                for ko in range(KD):
                    nc.tensor.matmul(ps[:], ff1w_t[ko][:, mo24 * P:(mo24 + 1) * P],
                                     x1nT[ko][:], start=(ko == 0), stop=(ko == KD - 1))
                if mo24 % 2 == 0:
                    nc.scalar.activation(hT[mo24][:], ps[:], AF.Relu,
                                         bias=ff1b_sb[:, i, mo24:mo24 + 1], scale=1.0)
                else:
                    nc.vector.tensor_scalar(hT[mo24][:], ps[:],
                                            ff1b_sb[:, i, mo24:mo24 + 1], 0.0,
                                            op0=OP.add, op1=OP.max)

            # ff2 + residual
            ps_y = [psW.tile([P, D], f32, tag="w", name="w") for _ in range(MT)]
            for ko in range(KF):
                for mo in range(MT):
                    for n0, n1 in ((0, 512), (512, D)):
                        nc.tensor.matmul(
                            ps_y[mo][:, n0:n1], hT[ko][:, mo * P:(mo + 1) * P],
                            ff2w_t[ko][:, n0:n1],
                            start=(ko == 0), stop=(ko == KF - 1))
            x2_t = [xpool.tile([P, D], f32, tag="x", name="x") for _ in range(MT)]
            for mo in range(MT):
                nc.vector.scalar_tensor_tensor(
                    x2_t[mo][:], ps_y[mo][:], 1.0, x1n_t[mo][:],
                    op0=OP.mult, op1=OP.add)
                if ff2b_nz:
                    nc.vector.tensor_add(x2_t[mo][:], x2_t[mo][:], ff2b_bc[:])

            x_t = layernorm(x2_t, ln2w_bc, ln2b_bc)
            if debug:
                for b in range(BL):
                    nc.sync.dma_start(out=xdbg_d[i + 1, b], in_=x_t[b][:])

        # ---- classifier ----
        xT = transpose_cast(x_t)
        ps_h = psS.tile([P, P * MT], f32, tag="s", name="s")
        for ko in range(KD):
            nc.tensor.matmul(ps_h[0:100, :], w1T_sb[:, ko, :], xT[ko][:],
                             start=(ko == 0), stop=(ko == KD - 1))
        hTa = const.tile([P, P * MT], bf16, tag="hTa", name="hTa")
        nc.vector.memset(hTa[:, :], 1.0)
        nc.vector.tensor_copy(hTa[0:100, :], ps_h[0:100, :])

        # logits chunks; evictions alternate Scalar/Vector, DMA from SBUF
        CH = 768
        for ci, c0 in enumerate(range(0, NE, CH)):
            cw = min(CH, NE - c0)
            w2t = w2p.tile([P, CH], bf16, tag="w2", name="w2")
            nc.sync.dma_start(out=w2t[:, 0:cw], in_=w2a_d[ci, :, 0:cw])
            for mo in range(MT):
                ps = psW.tile([P, CH], f32, tag="w", name="w")
                for s0 in range(0, cw, 512):
                    sw = min(512, cw - s0)
                    nc.tensor.matmul(ps[:, s0:s0 + sw],
                                     hTa[:, mo * P:(mo + 1) * P],
                                     w2t[:, s0:s0 + sw], start=True, stop=True)
                ost = ostp.tile([P, CH], f32, tag="ost", name="ost")
                if (ci + mo) % 2 == 0:
                    nc.scalar.copy(ost[:, 0:cw], ps[:, 0:cw])
                else:
                    nc.vector.tensor_copy(ost[:, 0:cw], ps[:, 0:cw])
                nc.sync.dma_start(out=out_d[mo, :, c0:c0 + cw], in_=ost[:, 0:cw])

    nc.compile()
    return nc


def _chunk_w2(cls_w2, cls_b2):
    # rows: 100 weights + 1 bias + 27 zero pad (lhsT rows 101.. are 1.0 from
    # the hTa memset, so the zero rows contribute nothing)
    w2a = np.concatenate(
        [cls_w2.T, cls_b2[None, :], np.zeros((27, NE), np.float32)], axis=0
    ).astype(BF16)  # [128, NE]
    nch = (NE + 767) // 768
    pad = nch * 768 - NE
    if pad:
        w2a = np.concatenate([w2a, np.zeros((128, pad), BF16)], axis=1)
    return np.ascontiguousarray(w2a.reshape(128, nch, 768).transpose(1, 0, 2))


def _prep(inputs):
    lhs = np.asarray(inputs["last_hidden_state"], dtype=np.float32)
    pos = np.asarray(inputs["entity_position_ids"])
    msk = np.asarray(inputs["entity_attention_mask"])
    qkv_w = np.asarray(inputs["qkv_w"], dtype=np.float32)
    qkv_b = np.asarray(inputs["qkv_b"], dtype=np.float32)
    out_w = np.asarray(inputs["out_w"], dtype=np.float32)
    out_b = np.asarray(inputs["out_b"], dtype=np.float32)
    ln1_w = np.asarray(inputs["ln1_w"], dtype=np.float32)
    ln1_b = np.asarray(inputs["ln1_b"], dtype=np.float32)
    ff1_w = np.asarray(inputs["ff1_w"], dtype=np.float32)
    ff1_b = np.asarray(inputs["ff1_b"], dtype=np.float32)
    ff2_w = np.asarray(inputs["ff2_w"], dtype=np.float32)
    ff2_b = np.asarray(inputs["ff2_b"], dtype=np.float32)
    ln2_w = np.asarray(inputs["ln2_w"], dtype=np.float32)
    ln2_b = np.asarray(inputs["ln2_b"], dtype=np.float32)
    cls_w1 = np.asarray(inputs["cls_w1"], dtype=np.float32)
    cls_w2 = np.asarray(inputs["cls_w2"], dtype=np.float32)
    cls_b2 = np.asarray(inputs["cls_b2"], dtype=np.float32)
    attn_w = np.asarray(inputs["attn_w"], dtype=np.float32)
    attn_b = float(np.asarray(inputs["attn_b"], dtype=np.float32))

    # ragged valid mask: 1 up to the first -1 (and only where attention mask set)
    nb = np.cumprod((pos != -1).astype(np.int32), axis=-1)
    valid = (msk != 0).astype(np.int32)[:, :, None] * nb       # [B, M, L]
    vmT = np.ascontiguousarray(valid.transpose(0, 2, 1)).astype(np.float32)

    cfg = (
        attn_b,
        bool(np.any(qkv_b[:, 2 * D:])),
        bool(np.any(out_b)),
        bool(np.any(ff2_b)),
        not (np.all(ln1_w == 1.0) and np.all(ln1_b == 0.0)),
        not (np.all(ln2_w == 1.0) and np.all(ln2_b == 0.0)),
        bool(KERNEL_DEBUG),
    )

    shared = {
        "attnw": attn_w,
        "qkvw": np.ascontiguousarray(qkv_w.transpose(0, 2, 1)).reshape(
            NL, KD, P, 3 * D).astype(BF16),
        "qkvb": qkv_b,
        "outw": np.ascontiguousarray(out_w.transpose(0, 2, 1)).reshape(
            NL, KD, P, D).astype(BF16),
        "ff1w": np.ascontiguousarray(ff1_w.transpose(0, 2, 1)).reshape(
            NL, KD, P, DFF).astype(BF16),
        "ff1b": ff1_b,
        "ff2w": np.ascontiguousarray(ff2_w.transpose(0, 2, 1)).reshape(
            NL, KF, P, D).astype(BF16),
        "w1T": np.ascontiguousarray(cls_w1.T).reshape(KD, P, 100).astype(BF16),
        "w2a": _chunk_w2(cls_w2, cls_b2),
    }
    if cfg[2]:
        shared["outb"] = out_b
    if cfg[3]:
        shared["ff2b"] = ff2_b
    if cfg[4]:
        shared["ln1w"] = ln1_w
        shared["ln1b"] = ln1_b
    if cfg[5]:
        shared["ln2w"] = ln2_w
        shared["ln2b"] = ln2_b

    lhs32 = np.ascontiguousarray(lhs[:, :L, :])
    in_maps = []
    for c in range(N_CORES):
        m = dict(shared)
        m["lhs32"] = np.ascontiguousarray(lhs32[c * BL:(c + 1) * BL])
        m["vmT"] = np.ascontiguousarray(vmT[c * BL:(c + 1) * BL])
        in_maps.append(m)
    return cfg, in_maps


def kernel(**inputs):
    from concourse.bass_utils import run_bass_kernel_spmd

    cfg, in_maps = _prep(inputs)
    if cfg not in _CACHE:
        _CACHE[cfg] = _build(cfg)
    nc = _CACHE[cfg]
    res = run_bass_kernel_spmd(nc, in_maps, core_ids=list(range(N_CORES)))
    out = np.concatenate([res.results[c]["out"] for c in range(N_CORES)], axis=0)
    if KERNEL_DEBUG:
        kernel.last_debug = [res.results[c].get("xdbg") for c in range(N_CORES)]
    return out
